# revision 27
# baseline (speedup 1.0000x reference)
"""Distributed GCN (2x GCNConv + Linear) on 8 Trainium2 NeuronCores via Bass/Tile.

Algorithm (matches the PyG-style reference):
  h1 = relu(gcnconv(x, W1, b1, mask1));  h2 = relu(gcnconv(h1, W2, b2, mask2))
  out = h2 @ Wl + bl
where gcnconv(x, W, b, keep) with self-loops:
  h = x @ W;  deg = segsum(keep, dst) + 1;  dis = rsqrt(deg)
  out = segsum(h[src] * (keep * dis[src] * dis[dst]), dst) + h * dis^2 + b

Distribution/schedule (v2 — gather-prep pipelined):
  * Layer 1: every core computes the FULL H1 = X@W1 (replicated, bf16) and
    writes it to local DRAM in global node order — no AllGather for layer 1.
    A small per-core pass also computes the core's own-shard H1 into SBUF for
    the self-loop term.
  * Edges are partitioned by dst core; per (dst-tile-group g, src-section s)
    they form a gather "run" (padded to the max block count over cores so the
    SPMD program is shape-uniform; edges sorted by (dst tile, src) for HBM
    locality).  dma_gather descriptor generation runs ahead with
    prepare_only=True (queue = src section, two groups of lookahead);
    trigger_dma(count=None) fires a queue's pending preps once that
    section's H rows are resident (layer 1: local stores; layer 2:
    AllGather of that section).  This keeps GPSIMD descriptor generation
    off the critical path of the collectives and data stores.
  * Aggregation: per dst tile, PSUM accumulates G_blk^T @ M_blk where M_blk
    is built on device by one VectorE tensor_scalar per block:
    M[p, d] = (iota[d] == dl[p]) * cf[p], from tiny per-edge dl/cf streams
    (dst-slot + normalization coef).  Self-loop blocks use the own-shard H
    tile scaled by dis^2 against an identity.  ReLU+bias runs on ScalarE in
    the transposed layout; the next layer's H-matmul follows per tile.
  * Layer 2 H2 tiles are stored to a local shard buffer; a 4-section
    AllGather (interleaved with the per-section triggers) makes them
    available for the layer-2 gathers.

Host-side numpy does graph preprocessing only (edge partitioning, padding,
degree/normalization scalars, index/dl/cf layout); all O(N*F) / O(E*F) float
work runs on the NeuronCores.
"""

import numpy as np
import ml_dtypes

import concourse.bass as bass
import concourse.bacc as bacc
import concourse.tile as tile
import concourse.mybir as mybir
from concourse.bass_utils import run_bass_kernel_spmd
from concourse.tile_sem_assignment import PROC_NAME_TO_IDX

N_SWDGE_LANES = 8   # tile's NUM_SWDGE_GLOBAL_SEMS (DMASW proc lanes)
PREP_TRIGGER = False  # False: immediate gathers (AllGathers hoisted before them)

P = 128
N_CORES = 8

N_NODES = 50000
F_IN = 128
F_HID = 128
F_OUT = 64

N_PAD = 50176
SHARD = N_PAD // N_CORES            # 6272
TILES_PC = SHARD // P               # 49
N_SEC = 4
SEC_L1 = N_PAD // N_SEC             # 12544 (global row sections)
SEC_L2 = SHARD // N_SEC             # 1568  (per-shard row sections)
# dst-tile groups (local tile ranges per core)
GROUP_BOUNDS = [0, 7, 14, 21, 28, 35, 42, TILES_PC]
N_GROUPS = len(GROUP_BOUNDS) - 1
LOOK = 2                            # groups of gather-prep lookahead
DL_PAD = 255.0                      # sentinel dst-slot: matches no iota column


# ---------------------------------------------------------------------------
# Host-side preprocessing
# ---------------------------------------------------------------------------

class _Run:
    __slots__ = ("g", "s", "nblk", "col0", "rid")

    def __init__(self, g, s, nblk, col0, rid):
        self.g, self.s, self.nblk, self.col0, self.rid = g, s, nblk, col0, rid


class _LayerLayout:
    __slots__ = ("runs", "run_by_gs", "n_builds", "idx_cols", "tile_sched")

    def __init__(self):
        self.runs = []            # group-major emission order
        self.run_by_gs = {}       # (g,s) -> _Run
        self.n_builds = 0
        self.idx_cols = 0
        # local tile -> [(rid, b_local, build_col), ...]
        self.tile_sched = {}


def _prep_layer(src_k, dst_k, coef_k, li):
    """Shared layout + per-core idx/dl/cf arrays for one layer."""
    src_k = src_k.astype(np.int64)
    dst_k = dst_k.astype(np.int64)
    if li == 0:
        sec = src_k // SEC_L1
        gidx = src_k % SEC_L1
    else:
        c_s = src_k // SHARD
        r = src_k % SHARD
        sec = r // SEC_L2
        gidx = c_s * SEC_L2 + (r % SEC_L2)
    core = dst_k // SHARD
    lt = (dst_k % SHARD) // P
    dl = dst_k % P
    gid = np.searchsorted(GROUP_BOUNDS, lt, side="right") - 1
    cf = coef_k.astype(np.float32)

    # per (core, g, s): sorted edge arrays + per-tile start offsets
    per_run = {}
    cnt = np.zeros((N_CORES, N_GROUPS, N_SEC), dtype=np.int64)
    for c in range(N_CORES):
        mc = core == c
        for g in range(N_GROUPS):
            t_lo, t_hi = GROUP_BOUNDS[g], GROUP_BOUNDS[g + 1]
            mg = mc & (gid == g)
            for s in range(N_SEC):
                m = mg & (sec == s)
                lt_r, gx_r, dl_r, cf_r = lt[m], gidx[m], dl[m], cf[m]
                o = np.lexsort((gx_r, lt_r))
                lt_r, gx_r, dl_r, cf_r = lt_r[o], gx_r[o], dl_r[o], cf_r[o]
                starts = np.searchsorted(lt_r, np.arange(t_lo, t_hi + 1))
                per_run[(c, g, s)] = (gx_r, dl_r, cf_r, starts)
                cnt[c, g, s] = len(gx_r)

    lay = _LayerLayout()
    col0 = 0
    rid = 0
    for g in range(N_GROUPS):
        for s in range(N_SEC):
            nblk = int(-(-cnt[:, g, s].max() // P))
            r = _Run(g, s, nblk, col0, rid)
            lay.runs.append(r)
            lay.run_by_gs[(g, s)] = r
            col0 += nblk * P // 16
            rid += 1
    lay.idx_cols = col0

    # shared build schedule: per tile, per sec, block span = union over cores
    j = 0
    builds = []   # (g, s, t, b, j)
    for g in range(N_GROUPS):
        t_lo, t_hi = GROUP_BOUNDS[g], GROUP_BOUNDS[g + 1]
        for t in range(t_lo, t_hi):
            sched = []
            for s in range(N_SEC):
                r = lay.run_by_gs[(g, s)]
                b0, b1 = 10 ** 9, 0
                for c in range(N_CORES):
                    starts = per_run[(c, g, s)][3]
                    a, b = starts[t - t_lo], starts[t - t_lo + 1]
                    if b > a:
                        b0 = min(b0, a // P)
                        b1 = max(b1, -(-b // P))
                for bb in range(b0, b1):
                    sched.append((r.rid, bb, j))
                    builds.append((g, s, t, bb, j))
                    j += 1
            lay.tile_sched[t] = sched
    lay.n_builds = j

    # per-core arrays
    per_core = []
    for c in range(N_CORES):
        idx16 = np.zeros((max(lay.idx_cols, 1) * 16,), dtype=np.int16)
        dla = np.full((P, max(j, 1)), DL_PAD, dtype=np.float32)
        cfa = np.zeros((P, max(j, 1)), dtype=np.float32)
        for r in lay.runs:
            gx_r = per_run[(c, r.g, r.s)][0]
            sl = slice(r.col0 * 16, r.col0 * 16 + len(gx_r))
            idx16[sl] = gx_r.astype(np.int16)
        for (g, s, t, bb, jj) in builds:
            gx_r, dl_r, cf_r, starts = per_run[(c, g, s)]
            t_lo = GROUP_BOUNDS[g]
            a, b = int(starts[t - t_lo]), int(starts[t - t_lo + 1])
            lo = max(a, bb * P)
            hi = min(b, (bb + 1) * P)
            if lo < hi:
                rows = np.arange(lo, hi)
                dla[rows - bb * P, jj] = dl_r[rows]
                cfa[rows - bb * P, jj] = cf_r[rows]
        w = idx16.reshape(-1, 16).T
        idxw = np.ascontiguousarray(np.tile(w, (8, 1)))
        per_core.append({"idx": idxw, "dl": dla, "cf": cfa})
    return lay, per_core


def _prepare(x, edge_index, mask1, mask2, W1, b1, W2, b2, Wl, bl,
             n, n_pad):
    assert n_pad == N_PAD
    bf16 = ml_dtypes.bfloat16
    src = np.asarray(edge_index[0], dtype=np.int64)
    dst = np.asarray(edge_index[1], dtype=np.int64)

    layouts = []
    layer_data = []
    selfws = []
    for li, mask in enumerate((np.asarray(mask1), np.asarray(mask2))):
        keep = mask.astype(bool)
        ks, kd = src[keep], dst[keep]
        deg = np.bincount(kd, minlength=n).astype(np.float64) + 1.0
        dis = 1.0 / np.sqrt(deg)
        coef_k = (dis[ks] * dis[kd]).astype(np.float32)
        selfw = np.zeros((n_pad,), dtype=np.float32)
        selfw[:n] = (dis * dis).astype(np.float32)
        lay, pc = _prep_layer(ks, kd, coef_k, li)
        layouts.append(lay)
        layer_data.append(pc)
        selfws.append(selfw)

    xp = np.zeros((n_pad, F_IN), dtype=np.float32)
    xp[:n] = np.asarray(x, dtype=np.float32)
    xt_full = np.ascontiguousarray(xp.T.astype(bf16))

    ident = np.eye(P, dtype=np.float32)
    iota = np.broadcast_to(np.arange(P, dtype=np.float32), (P, P)).copy()

    in_maps = []
    for c in range(N_CORES):
        m = {
            "xt": xt_full,
            "xto": np.ascontiguousarray(
                xp[c * SHARD:(c + 1) * SHARD].T.astype(bf16)),
            "w1": np.asarray(W1, np.float32).astype(bf16),
            "w2": np.asarray(W2, np.float32).astype(bf16),
            "wl": np.asarray(Wl, np.float32).astype(bf16),
            "b1c": np.asarray(b1, np.float32).reshape(P, 1),
            "b2c": np.asarray(b2, np.float32).reshape(P, 1),
            "blbc": np.broadcast_to(np.asarray(bl, np.float32),
                                    (P, F_OUT)).copy(),
            "ident": ident.astype(bf16),
            "iota": iota.astype(bf16),
        }
        for li in (0, 1):
            d = layer_data[li][c]
            m[f"idx{li+1}"] = d["idx"]
            m[f"dl{li+1}"] = d["dl"].astype(bf16)
            m[f"cf{li+1}"] = d["cf"].astype(bf16)
            sw = selfws[li][c * SHARD:(c + 1) * SHARD]
            m[f"sw{li+1}"] = np.ascontiguousarray(
                sw.reshape(TILES_PC, P).T.astype(np.float32))
        in_maps.append(m)
    return layouts, in_maps


# ---------------------------------------------------------------------------
# Device program
# ---------------------------------------------------------------------------

def _build(layouts, n_pad):
    assert n_pad == N_PAD
    gdt = mybir.dt.bfloat16
    f32 = mybir.dt.float32

    nc = bacc.Bacc("TRN2", target_bir_lowering=False, debug=False,
                   num_swdge_queues=N_SEC)

    xt_d = nc.declare_dram_parameter("xt", [P, N_PAD], gdt, isOutput=False)
    xto_d = nc.declare_dram_parameter("xto", [P, SHARD], gdt, isOutput=False)
    w1_d = nc.declare_dram_parameter("w1", [P, F_HID], gdt, isOutput=False)
    w2_d = nc.declare_dram_parameter("w2", [P, F_HID], gdt, isOutput=False)
    wl_d = nc.declare_dram_parameter("wl", [P, F_OUT], gdt, isOutput=False)
    b1c_d = nc.declare_dram_parameter("b1c", [P, 1], f32, isOutput=False)
    b2c_d = nc.declare_dram_parameter("b2c", [P, 1], f32, isOutput=False)
    blbc_d = nc.declare_dram_parameter("blbc", [P, F_OUT], f32, isOutput=False)
    ident_d = nc.declare_dram_parameter("ident", [P, P], gdt, isOutput=False)
    iota_d = nc.declare_dram_parameter("iota", [P, P], gdt, isOutput=False)
    idx_d, dl_d, cf_d, sw_d = [], [], [], []
    for li, lay in enumerate(layouts):
        ic = max(lay.idx_cols, 1)
        nb = max(lay.n_builds, 1)
        idx_d.append(nc.declare_dram_parameter(
            f"idx{li+1}", [P, ic], mybir.dt.int16, isOutput=False))
        dl_d.append(nc.declare_dram_parameter(
            f"dl{li+1}", [P, nb], gdt, isOutput=False))
        cf_d.append(nc.declare_dram_parameter(
            f"cf{li+1}", [P, nb], gdt, isOutput=False))
        sw_d.append(nc.declare_dram_parameter(
            f"sw{li+1}", [P, TILES_PC], f32, isOutput=False))
    out_d = nc.declare_dram_parameter("out", [SHARD, F_OUT], f32, isOutput=True)

    h1_sec = nc.dram_tensor("h1_sec", [N_PAD, P], gdt)
    h2_shard = nc.dram_tensor("h2_shard", [SHARD, P], gdt)
    h2_sec = [nc.dram_tensor(f"h2_sec{s}", [N_CORES * SEC_L2, P], gdt,
                             addr_space="Shared") for s in range(N_SEC)]

    rg = [list(range(N_CORES))]
    relu = mybir.ActivationFunctionType.Relu
    copyf = mybir.ActivationFunctionType.Copy
    max_run_nb = max((r.nblk for lay in layouts for r in lay.runs), default=1)
    max_tile_b = max((len(s) for lay in layouts for s in
                      lay.tile_sched.values()), default=1)
    # phase-0 streaming: 14 chunks of 28 tiles (3584 nodes) each
    XCH = 14
    XCH_T = N_PAD // (XCH * P)      # 28

    # Per-DMASW-lane completion semaphores.  Tile's managed path pre-bumps
    # its DMASW lane sems at prep time (descriptor-write), so data consumers
    # wired to DMASW would not wait for the triggered DMA.  We pass our own
    # sem per prep (fixed after scheduling to match the prep's DMASW lane)
    # and retarget all DMASW waits to these sems post-scheduling.
    glane = [nc.alloc_semaphore(f"glane{j}") for j in range(N_SWDGE_LANES)]
    prep_ctr = [0]
    lane_cnt = [0] * N_SWDGE_LANES
    run_wait = {}        # (li, rid) -> (sem, cumulative target)

    with tile.TileContext(nc) as tc:
        with (
            tc.tile_pool(name="consts", bufs=1) as cpool,
            tc.tile_pool(name="xs", bufs=2) as xpool,
            tc.tile_pool(name="hstage", bufs=2) as stpool,
            tc.tile_pool(name="gbuf", bufs=LOOK * N_SEC) as gpool,
            tc.tile_pool(name="mpool", bufs=3) as mpool,
            tc.tile_pool(name="spool", bufs=4) as spool,
            tc.tile_pool(name="opool", bufs=8) as opool,
            tc.tile_pool(name="aggp", bufs=4, space="PSUM") as aggpool,
            tc.tile_pool(name="hp", bufs=4, space="PSUM") as hpool,
        ):
            def load_const(dram, shape, dt):
                t = cpool.tile(shape, dt, tag=dram.name)
                nc.sync.dma_start(t[:], dram[:])
                return t

            xto_sb = load_const(xto_d, [P, SHARD], gdt)
            w1_sb = load_const(w1_d, [P, F_HID], gdt)
            w2_sb = load_const(w2_d, [P, F_HID], gdt)
            wl_sb = load_const(wl_d, [P, F_OUT], gdt)
            b1c_sb = load_const(b1c_d, [P, 1], f32)
            b2c_sb = load_const(b2c_d, [P, 1], f32)
            blbc_sb = load_const(blbc_d, [P, F_OUT], f32)
            ident_sb = load_const(ident_d, [P, P], gdt)
            iota_sb = load_const(iota_d, [P, P], gdt)
            idx_sb = [load_const(idx_d[li], [P, max(layouts[li].idx_cols, 1)],
                                 mybir.dt.int16) for li in (0, 1)]
            dl_sb = [load_const(dl_d[li], [P, max(layouts[li].n_builds, 1)],
                                gdt) for li in (0, 1)]
            cf_sb = [load_const(cf_d[li], [P, max(layouts[li].n_builds, 1)],
                                gdt) for li in (0, 1)]
            sw_sb = [load_const(sw_d[li], [P, TILES_PC], f32) for li in (0, 1)]
            # own-shard H kept in SBUF for the self-loop term
            h_own = [cpool.tile([P, TILES_PC, P], gdt, tag=f"h{li}own",
                                name=f"h{li}own")
                     for li in (1, 2)]

            # ---- phase 0a: own-shard H1 (for self-loops) ----
            for t in range(TILES_PC):
                hp = hpool.tile([P, F_HID], f32, tag="hpsum")
                nc.tensor.matmul(out=hp[:], lhsT=xto_sb[:, t * P:(t + 1) * P],
                                 rhs=w1_sb[:], start=True, stop=True)
                nc.scalar.activation(out=h_own[0][:, t, :], in_=hp[:],
                                     func=copyf)

            # ---- phase 0b: full H1 (replicated), streamed + bulk stores ----
            for ch in range(XCH):
                xts = xpool.tile([P, XCH_T * P], gdt, tag="xts")
                nc.sync.dma_start(
                    xts[:], xt_d[:, ch * XCH_T * P:(ch + 1) * XCH_T * P])
                hst = stpool.tile([P, XCH_T, P], gdt, tag="hst")
                for k in range(XCH_T):
                    hp = hpool.tile([P, F_HID], f32, tag="hpsum")
                    nc.tensor.matmul(out=hp[:], lhsT=xts[:, k * P:(k + 1) * P],
                                     rhs=w1_sb[:], start=True, stop=True)
                    nc.scalar.activation(out=hst[:, k, :], in_=hp[:],
                                         func=copyf)
                rows = slice(ch * XCH_T * P, (ch + 1) * XCH_T * P)
                nc.sync.dma_start(
                    h1_sec[rows, :].rearrange("(k p) f -> p k f", p=P),
                    hst[:])

            # ---- gather preps + triggers + aggregation per layer ----
            gb_tiles = [{}, {}]

            def emit_preps(li, g):
                lay = layouts[li]
                for s in range(N_SEC):
                    r = lay.run_by_gs[(g, s)]
                    if r.nblk == 0:
                        continue
                    gb = gpool.tile([P, max_run_nb, P], gdt, tag="gb")
                    ni = r.nblk * P
                    if li == 0:
                        src_ap = h1_sec[r.s * SEC_L1:(r.s + 1) * SEC_L1, :]
                    else:
                        src_ap = h2_sec[r.s][:]
                    if PREP_TRIGGER:
                        lane = prep_ctr[0] % N_SWDGE_LANES
                        nc.gpsimd.dma_gather(
                            gb[:, :r.nblk, :], src_ap,
                            idx_sb[li][:, r.col0:r.col0 + ni // 16],
                            ni, ni, P, single_packet=False,
                            prepare_only=True,
                            sem=glane[lane],
                            queue_num=r.s)
                        prep_ctr[0] += 1
                        lane_cnt[lane] += 1
                        run_wait[(li, r.rid)] = (glane[lane],
                                                 16 * lane_cnt[lane])
                    else:
                        nc.gpsimd.dma_gather(
                            gb[:, :r.nblk, :], src_ap,
                            idx_sb[li][:, r.col0:r.col0 + ni // 16],
                            ni, ni, P, single_packet=False,
                            queue_num=r.s)
                    gb_tiles[li][r.rid] = gb

            for li in (0, 1):
                lay = layouts[li]
                if not PREP_TRIGGER and li == 1:
                    for s in range(N_SEC):
                        nc.gpsimd.collective_compute(
                            "AllGather", mybir.AluOpType.bypass,
                            replica_groups=rg,
                            ins=[h2_shard[s * SEC_L2:(s + 1) * SEC_L2, :]],
                            outs=[h2_sec[s][:]])
                for g in range(min(LOOK, N_GROUPS)):
                    emit_preps(li, g)
                # fire the first LOOK groups (layer 2: after each section's
                # AllGather)
                if PREP_TRIGGER:
                    for s in range(N_SEC):
                        if li == 1:
                            nc.gpsimd.collective_compute(
                                "AllGather", mybir.AluOpType.bypass,
                                replica_groups=rg,
                                ins=[h2_shard[s * SEC_L2:(s + 1) * SEC_L2, :]],
                                outs=[h2_sec[s][:]])
                        nc.gpsimd.trigger_dma(count=None, queue_num=s)

                bcol = b1c_sb if li == 0 else b2c_sb
                w_next = w2_sb if li == 0 else wl_sb
                n_next = F_HID if li == 0 else F_OUT
                for g in range(N_GROUPS):
                    if g + LOOK < N_GROUPS:
                        emit_preps(li, g + LOOK)
                        if PREP_TRIGGER:
                            for s in range(N_SEC):
                                nc.gpsimd.trigger_dma(count=None, queue_num=s)
                    if PREP_TRIGGER and not globals().get("NO_BARRIER"):
                        # gate this group's G-consuming matmuls on the real
                        # gather completions (tile's own DMASW waits are
                        # satisfied at prep time by its pre-bumps)
                        for s in range(N_SEC):
                            w = run_wait.get((li, lay.run_by_gs[(g, s)].rid))
                            if w is not None:
                                nc.tensor.wait_ge(w[0], w[1])
                    t_lo, t_hi = GROUP_BOUNDS[g], GROUP_BOUNDS[g + 1]
                    aggp = None
                    for k, t in enumerate(range(t_lo, t_hi)):
                        if k % 4 == 0:
                            aggp = aggpool.tile([P, 512], f32, tag="aggp")
                        sl = slice((k % 4) * P, (k % 4) * P + P)
                        sched = lay.tile_sched[t]
                        nbb = len(sched)
                        first = True
                        if nbb:
                            # batched M build: all of this tile's blocks in
                            # two DVE ops (build cols are consecutive)
                            j0 = sched[0][2]
                            assert [x[2] for x in sched] == list(
                                range(j0, j0 + nbb))
                            mbt = mpool.tile([P, max_tile_b, P], gdt,
                                             tag="mt")
                            dl_b = dl_sb[li][:, j0:j0 + nbb].unsqueeze(
                                2).broadcast_to([P, nbb, P])
                            io_b = iota_sb[:].unsqueeze(1).broadcast_to(
                                [P, nbb, P])
                            cf_b = cf_sb[li][:, j0:j0 + nbb].unsqueeze(
                                2).broadcast_to([P, nbb, P])
                            nc.vector.tensor_tensor(
                                out=mbt[:, :nbb, :], in0=dl_b, in1=io_b,
                                op=mybir.AluOpType.is_equal)
                            nc.vector.tensor_tensor(
                                out=mbt[:, :nbb, :], in0=mbt[:, :nbb, :],
                                in1=cf_b, op=mybir.AluOpType.mult)
                            for bk, (rid, bb, jj) in enumerate(sched):
                                gb = gb_tiles[li][rid]
                                nc.tensor.matmul(out=aggp[:, sl],
                                                 lhsT=gb[:, bb, :],
                                                 rhs=mbt[:, bk, :],
                                                 start=first, stop=False)
                                first = False
                        gss = spool.tile([P, P], gdt, tag="gselfs")
                        nc.scalar.activation(out=gss[:], in_=h_own[li][:, t, :],
                                             func=copyf,
                                             scale=sw_sb[li][:, t:t + 1])
                        nc.tensor.matmul(out=aggp[:, sl], lhsT=gss[:],
                                         rhs=ident_sb[:], start=first,
                                         stop=True)
                        outT = opool.tile([P, P], gdt, tag="outT")
                        nc.scalar.activation(out=outT[:], in_=aggp[:, sl],
                                             func=relu, bias=bcol[:])
                        hp2 = hpool.tile([P, n_next], f32, tag="hpsum")
                        nc.tensor.matmul(out=hp2[:], lhsT=outT[:],
                                         rhs=w_next[:], start=True, stop=True)
                        rows = slice(t * P, (t + 1) * P)
                        if li == 0:
                            nc.scalar.activation(out=h_own[1][:, t, :],
                                                 in_=hp2[:], func=copyf)
                            hsb = opool.tile([P, n_next], gdt, tag="hsb")
                            nc.scalar.activation(out=hsb[:], in_=hp2[:],
                                                 func=copyf)
                            nc.sync.dma_start(h2_shard[rows, :], hsb[:])
                        else:
                            osb = opool.tile([P, F_OUT], f32, tag="osb")
                            nc.vector.tensor_tensor(
                                out=osb[:], in0=hp2[:], in1=blbc_sb[:],
                                op=mybir.AluOpType.add)
                            nc.sync.dma_start(out_d[rows, :], osb[:])

    _patch_swdge_waits(nc)
    nc.compile()
    return nc


def _patch_swdge_waits(nc):
    """Verify each prep's DMASW lane matches the emission-order rotation.

    Tile pre-bumps its DMASW lane sems at prep time for gen_mode==1 SWDGE
    preps, so its scheduled consumer waits are vacuous; real data gating is
    done by explicit nc.tensor.wait_ge barriers on the per-lane glane sems
    (bumped by the gather descriptors).  The barrier targets assume prep k
    sits on lane k%8 with cumulative +16 per prep — assert that here.
    """
    idx_to_proc = {v: k for k, v in PROC_NAME_TO_IDX.items()}
    insts = [i for blk in nc.m.functions[0].blocks for i in blk.instructions]
    sem_ids = {}
    for inst in insts:
        si = inst.sync_info
        if si is None:
            continue
        for u in si.on_update:
            if u.ant_name and u.ant_name.startswith("glane"):
                sem_ids[u.ant_name] = u.id
    k = 0
    for inst in insts:
        if type(inst).__name__ == "InstDMAGatherAnt" and inst.gen_mode == 1:
            lane = idx_to_proc[inst.bass_scheduled_proc]
            assert lane == f"DMASW{k % N_SWDGE_LANES}", (lane, k)
            nm = f"glane{lane[5:]}"
            u0 = inst.sync_info.on_update[0]
            assert u0.ant_name == nm, (u0.ant_name, nm)
            k += 1


# ---------------------------------------------------------------------------
# Entry point
# ---------------------------------------------------------------------------

def _run(x, edge_index, mask1, mask2, W1, b1, W2, b2, Wl, bl, n, n_pad):
    layouts, in_maps = _prepare(x, edge_index, mask1, mask2,
                                W1, b1, W2, b2, Wl, bl, n, n_pad)
    nc = _build(layouts, n_pad)
    res = run_bass_kernel_spmd(nc, in_maps, core_ids=list(range(N_CORES)))
    out = np.concatenate([res.results[c]["out"] for c in range(N_CORES)],
                         axis=0)
    return out[:n].astype(np.float32)


def kernel(x, edge_index, mask1, mask2, W1, b1, W2, b2, Wl, bl):
    return _run(x, edge_index, mask1, mask2, W1, b1, W2, b2, Wl, bl,
                N_NODES, N_PAD)


# revision 28
# speedup vs baseline: 1.0400x; 1.0400x over previous
"""Distributed GCN (2x GCNConv + Linear) on 8 Trainium2 NeuronCores via Bass/Tile.

Algorithm (matches the PyG-style reference):
  h1 = relu(gcnconv(x, W1, b1, mask1));  h2 = relu(gcnconv(h1, W2, b2, mask2))
  out = h2 @ Wl + bl
where gcnconv(x, W, b, keep) with self-loops:
  h = x @ W;  deg = segsum(keep, dst) + 1;  dis = rsqrt(deg)
  out = segsum(h[src] * (keep * dis[src] * dis[dst]), dst) + h * dis^2 + b

Distribution: nodes padded to N_PAD = 8 * SHARD, contiguous node shard per
core.  Edges partitioned by dst core.  Per layer: each core computes H for
its shard (TensorE), AllGather makes full H available in every core's DRAM
(bf16), then per 128-node dst tile the core bulk-gathers H[src] rows with
dma_gather (edge-major layout, round-robin over the 4 SWDGE queues so
descriptor generation pipelines across Q7 core pairs), folds the edge
coefficients into G with one broadcast tensor_tensor per chunk, builds
one-hot "segment matrices" M[e, d] = (dstloc[e] == d) in batches of 8
blocks with a single broadcast is_equal, and accumulates
out^T[f, d] += G_blk^T @ M_blk on TensorE in PSUM.  Self-loop blocks skip
the gather entirely: their H rows are the core's own shard rows (plain
affine DMA), scaled by dis^2 on ScalarE, matmul'd against an identity.
ReLU+bias runs on ScalarE straight out of PSUM (bias is per-partition in
the transposed layout), and the next layer's H-matmul follows per tile.

The int16 gather-index limit (32768 rows) is handled by splitting each
tile's edges into lo/hi halves by src and gathering from two base offsets.

Host-side numpy does graph preprocessing only (edge partitioning, padding,
degree/normalization scalars, index layout); all O(N*F) / O(E*F) float
work runs on the NeuronCores.
"""

import numpy as np
import ml_dtypes

import concourse.bass as bass
import concourse.bacc as bacc
import concourse.tile as tile
import concourse.mybir as mybir
from concourse.bass_utils import run_bass_kernel_spmd

P = 128
N_CORES = 8

# Full-problem dimensions (hardcoded per the task contract).
N_NODES = 50000
F_IN = 128
F_HID = 128
F_OUT = 64

# bf16 for gathered features / segment matrices (f32 PSUM accumulate).
GATHER_BF16 = True

# Gather chunking: one dma_gather covers <= SUB_B 128-edge blocks.
SUB_B = 24
# Tiles per compute group (gathers batched per group+half).
GROUP_T = 8
# SWDGE queues to rotate gathers over (4 Q7 core pairs).
N_QUEUES = 4
# src sections per shard (pipelined AllGather + int16 idx range).
N_SEC = 2


# ---------------------------------------------------------------------------
# Host-side preprocessing
# ---------------------------------------------------------------------------

class _Chunk:
    __slots__ = ("half", "nblk", "blk0", "col0", "segs")

    def __init__(self, half, nblk, blk0, col0):
        self.half = half
        self.nblk = nblk
        self.blk0 = blk0          # global block offset (dl/cf column)
        self.col0 = col0          # idx16 column offset
        self.segs = []            # (tile_pc, j0, nb) local block ranges


class _Group:
    __slots__ = ("tiles",)

    def __init__(self):
        # tile_pc -> [(chunk_idx, j0, nb), ...] in lo-then-hi order
        self.tiles = {}


class _LayerLayout:
    __slots__ = ("chunks", "groups", "n_blocks", "idx_cols")

    def __init__(self):
        self.chunks = []
        self.groups = []
        self.n_blocks = 0
        self.idx_cols = 0


def _prep_layer(src_k, dst_k, coef_k, n_pad, shard, group_t, sub_b):
    """Build the shared static layout + per-core device arrays for one layer.

    src_k/dst_k/coef_k: kept (mask=1) edges (self-loops handled separately).
    Returns (_LayerLayout, per_core list of dicts with idx16/dstloc/coef).
    """
    tiles_pc = shard // P
    n_tiles = n_pad // P

    sec_rows = shard // N_SEC
    s_all = src_k
    d_all = dst_k
    c_all = coef_k.astype(np.float32)

    tile_g = d_all // P                       # global dst tile
    half = (s_all % shard) // sec_rows        # src section within owner shard
    key = tile_g * N_SEC + half
    order = np.argsort(key, kind="stable")
    s_all, d_all, c_all, key = s_all[order], d_all[order], c_all[order], key[order]
    # section-space gather index: core*sec_rows + offset within section
    s_idx = (s_all // shard) * sec_rows + (s_all % shard) % sec_rows
    # boundaries of each (tile, section) bucket in the sorted arrays
    bnd = np.searchsorted(key, np.arange(N_SEC * n_tiles + 1))

    # raw counts per (core, tile_pc, half)
    cnt = np.zeros((N_CORES, tiles_pc, N_SEC), dtype=np.int64)
    for t in range(n_tiles):
        c, tt = divmod(t, tiles_pc)
        for h in range(N_SEC):
            cnt[c, tt, h] = bnd[N_SEC * t + h + 1] - bnd[N_SEC * t + h]
    # shared (max-over-cores) padded block counts
    bcnt = -(-cnt.max(axis=0) // P)           # [tiles_pc, 2] ceil-div

    lay = _LayerLayout()
    blk0 = 0
    col0 = 0
    for g0 in range(0, tiles_pc, group_t):
        g_tiles = range(g0, min(g0 + group_t, tiles_pc))
        grp = _Group()
        for tt in g_tiles:
            grp.tiles[tt] = []
        for h in range(N_SEC):
            ck = None
            for tt in g_tiles:
                nb = int(bcnt[tt, h])
                if nb == 0:
                    continue
                if ck is None or ck.nblk + nb > sub_b:
                    ck = _Chunk(h, 0, blk0, col0)
                    lay.chunks.append(ck)
                ck.segs.append((tt, ck.nblk, nb))
                grp.tiles[tt].append((len(lay.chunks) - 1, ck.nblk, nb))
                ck.nblk += nb
                blk0 += nb
                col0 += nb * P // 16
        lay.groups.append(grp)
    lay.n_blocks = blk0
    lay.idx_cols = col0

    # per-core data arrays in the exact chunk/block order above.
    # M is the normalized adjacency in block-one-hot form, built on host:
    # block b, edge row p -> M[p, b*128 + dstloc] = coef.
    per_core = []
    for c in range(N_CORES):
        idx16 = np.zeros((max(lay.idx_cols, 1) * 16,), dtype=np.int16)
        mbig = np.zeros((P, max(lay.n_blocks, 1) * P), dtype=np.float32)
        for ck in lay.chunks:
            for (tt, j0, nb) in ck.segs:
                t = c * tiles_pc + tt
                a, b = bnd[N_SEC * t + ck.half], bnd[N_SEC * t + ck.half + 1]
                n_e = b - a
                assert n_e <= nb * P
                src_t = s_idx[a:b]
                dl_t = (d_all[a:b] % P).astype(np.int64)
                cf_t = c_all[a:b]
                # flat edge slots for this (tile,sec): blocks j0..j0+nb of ck
                e0 = (ck.blk0 + j0) * P
                idx_flat_base = ck.col0 * 16 - ck.blk0 * P
                sl = slice(idx_flat_base + e0, idx_flat_base + e0 + n_e)
                idx16[sl] = src_t.astype(np.int16)
                eloc = np.arange(n_e)
                bcol = (ck.blk0 + j0) + eloc // P
                prow = eloc % P
                mbig[prow, bcol * P + dl_t] = cf_t
        # wrap idx16 into [128, idx_cols] (16-part wrap, replicated x8)
        w = idx16.reshape(-1, 16).T                      # [16, idx_cols]
        idxw = np.ascontiguousarray(np.tile(w, (8, 1)))
        per_core.append({"idx": idxw, "m": mbig})
    return lay, per_core


def _prepare(x, edge_index, mask1, mask2, W1, b1, W2, b2, Wl, bl,
             n, n_pad, group_t=GROUP_T, sub_b=SUB_B):
    """Full host prep: returns (static_layouts, in_maps)."""
    shard = n_pad // N_CORES
    tiles_pc = shard // P
    assert shard % P == 0
    src = np.asarray(edge_index[0], dtype=np.int64)
    dst = np.asarray(edge_index[1], dtype=np.int64)

    np_g = ml_dtypes.bfloat16 if GATHER_BF16 else np.float32

    layouts = []
    layer_data = []
    selfws = []
    for mask in (np.asarray(mask1), np.asarray(mask2)):
        keep = mask.astype(bool)
        ks, kd = src[keep], dst[keep]
        deg = np.bincount(kd, minlength=n).astype(np.float64) + 1.0
        dis = 1.0 / np.sqrt(deg)
        coef_k = (dis[ks] * dis[kd]).astype(np.float32)
        selfw = np.zeros((n_pad,), dtype=np.float32)
        selfw[:n] = (dis * dis).astype(np.float32)
        lay, pc = _prep_layer(ks, kd, coef_k, n_pad, shard,
                              group_t, sub_b)
        layouts.append(lay)
        layer_data.append(pc)
        selfws.append(selfw)

    xp = np.zeros((n_pad, F_IN), dtype=np.float32)
    xp[:n] = np.asarray(x, dtype=np.float32)

    ident = np.eye(P, dtype=np.float32)

    in_maps = []
    for c in range(N_CORES):
        m = {
            "xt": np.ascontiguousarray(xp[c * shard:(c + 1) * shard].T),
            "w1": np.asarray(W1, np.float32),
            "w2": np.asarray(W2, np.float32),
            "wl": np.asarray(Wl, np.float32),
            "b1c": np.asarray(b1, np.float32).reshape(P, 1),
            "b2c": np.asarray(b2, np.float32).reshape(P, 1),
            "blbc": np.broadcast_to(np.asarray(bl, np.float32),
                                    (P, F_OUT)).copy(),
            "ident": ident.astype(np_g),
        }
        for li in (0, 1):
            d = layer_data[li][c]
            m[f"idx{li+1}"] = d["idx"]
            m[f"m{li+1}"] = d["m"].astype(np_g)
            # selfw for this core's tiles: [128, tiles_pc] f32
            sw = selfws[li][c * shard:(c + 1) * shard]
            m[f"sw{li+1}"] = np.ascontiguousarray(
                sw.reshape(tiles_pc, P).T.astype(np.float32))
        in_maps.append(m)
    return layouts, in_maps


# ---------------------------------------------------------------------------
# Device program
# ---------------------------------------------------------------------------

def _build(layouts, n_pad):
    shard = n_pad // N_CORES
    tiles_pc = shard // P
    gdt = mybir.dt.bfloat16 if GATHER_BF16 else mybir.dt.float32
    f32 = mybir.dt.float32

    nc = bacc.Bacc("TRN2", target_bir_lowering=False, debug=False,
                   num_swdge_queues=N_QUEUES)

    xt_d = nc.declare_dram_parameter("xt", [P, shard], f32, isOutput=False)
    w1_d = nc.declare_dram_parameter("w1", [P, F_HID], f32, isOutput=False)
    w2_d = nc.declare_dram_parameter("w2", [P, F_HID], f32, isOutput=False)
    wl_d = nc.declare_dram_parameter("wl", [P, F_OUT], f32, isOutput=False)
    b1c_d = nc.declare_dram_parameter("b1c", [P, 1], f32, isOutput=False)
    b2c_d = nc.declare_dram_parameter("b2c", [P, 1], f32, isOutput=False)
    blbc_d = nc.declare_dram_parameter("blbc", [P, F_OUT], f32, isOutput=False)
    ident_d = nc.declare_dram_parameter("ident", [P, P], gdt, isOutput=False)
    idx_d, m_d, sw_d = [], [], []
    for li, lay in enumerate(layouts):
        ic = max(lay.idx_cols, 1)
        nb = max(lay.n_blocks, 1)
        idx_d.append(nc.declare_dram_parameter(
            f"idx{li+1}", [P, ic], mybir.dt.int16, isOutput=False))
        m_d.append(nc.declare_dram_parameter(
            f"m{li+1}", [P, nb * P], gdt, isOutput=False))
        sw_d.append(nc.declare_dram_parameter(
            f"sw{li+1}", [P, tiles_pc], f32, isOutput=False))
    out_d = nc.declare_dram_parameter("out", [shard, F_OUT], f32, isOutput=True)

    sec_rows = shard // N_SEC
    h_shard = [nc.dram_tensor(f"h{li}_shard", [shard, P], gdt)
               for li in (1, 2)]
    h_sec = [[nc.dram_tensor(f"h{li}_sec{s}", [N_CORES * sec_rows, P], gdt,
                             addr_space="Shared") for s in range(N_SEC)]
             for li in (1, 2)]

    rg = [list(range(N_CORES))]
    relu = mybir.ActivationFunctionType.Relu
    copyf = mybir.ActivationFunctionType.Copy
    max_chunk_nb = max((ck.nblk for lay in layouts for ck in lay.chunks),
                      default=1)
    qctr = [0]
    # first block / block count per group (for the per-group M stream)
    def group_span(lay, grp):
        cis = sorted({ci for segs in grp.tiles.values() for (ci, _, _) in segs})
        b0 = min(lay.chunks[ci].blk0 for ci in cis)
        b1 = max(lay.chunks[ci].blk0 + lay.chunks[ci].nblk for ci in cis)
        return b0, b1
    max_group_nb = max((group_span(lay, grp)[1] - group_span(lay, grp)[0]
                        for lay in layouts for grp in lay.groups), default=1)

    with tile.TileContext(nc) as tc:
        with (
            tc.tile_pool(name="consts", bufs=1) as cpool,
            tc.tile_pool(name="gbuf", bufs=13) as gpool,
            tc.tile_pool(name="mpool", bufs=3) as mpool,
            tc.tile_pool(name="spool", bufs=8) as spool,
            tc.tile_pool(name="opool", bufs=6) as opool,
            tc.tile_pool(name="aggp", bufs=5, space="PSUM") as aggpool,
            tc.tile_pool(name="hp", bufs=3, space="PSUM") as hpool,
        ):
            def load_const(dram, shape, dt):
                t = cpool.tile(shape, dt, tag=dram.name)
                nc.sync.dma_start(t[:], dram[:])
                return t

            xt_sb = load_const(xt_d, [P, shard], f32)
            w1_sb = load_const(w1_d, [P, F_HID], f32)
            w2_sb = load_const(w2_d, [P, F_HID], f32)
            wl_sb = load_const(wl_d, [P, F_OUT], f32)
            b1c_sb = load_const(b1c_d, [P, 1], f32)
            b2c_sb = load_const(b2c_d, [P, 1], f32)
            blbc_sb = load_const(blbc_d, [P, F_OUT], f32)
            ident_sb = load_const(ident_d, [P, P], gdt)
            idx_sb = [load_const(idx_d[li], [P, max(layouts[li].idx_cols, 1)],
                                 mybir.dt.int16) for li in (0, 1)]
            sw_sb = [load_const(sw_d[li], [P, tiles_pc], f32) for li in (0, 1)]

            # ---- phase 0: H1 = X @ W1 (per-shard), sectioned AllGather ----
            for tt in range(tiles_pc):
                hp = hpool.tile([P, F_HID], f32, tag="hpsum")
                nc.tensor.matmul(out=hp[:], lhsT=xt_sb[:, tt * P:(tt + 1) * P],
                                 rhs=w1_sb[:], start=True, stop=True)
                hsb = opool.tile([P, F_HID], gdt, tag="hsb")
                nc.scalar.activation(out=hsb[:], in_=hp[:], func=copyf)
                nc.sync.dma_start(h_shard[0][tt * P:(tt + 1) * P, :], hsb[:])
                for s in range(N_SEC):
                    if tt * P < (s + 1) * sec_rows <= (tt + 1) * P:
                        nc.gpsimd.collective_compute(
                            "AllGather", mybir.AluOpType.bypass,
                            replica_groups=rg,
                            ins=[h_shard[0][s * sec_rows:(s + 1) * sec_rows, :]],
                            outs=[h_sec[0][s][:]])

            # ---- aggregation layers ----
            for li in (0, 1):
                lay = layouts[li]
                bcol = b1c_sb if li == 0 else b2c_sb
                w_next = w2_sb if li == 0 else wl_sb
                n_next = F_HID if li == 0 else F_OUT

                for gi, grp in enumerate(lay.groups):
                    # stream this group's M panel + issue its gathers
                    gb0, gb1 = group_span(lay, grp)
                    mw = mpool.tile([P, max_group_nb * P], gdt, tag="mw")
                    nc.sync.dma_start(mw[:, :(gb1 - gb0) * P],
                                      m_d[li][:, gb0 * P:gb1 * P])
                    need = sorted({ci for segs in grp.tiles.values()
                                   for (ci, _, _) in segs})
                    gbufs = {}
                    for ci in need:
                        ck = lay.chunks[ci]
                        gb = gpool.tile([P, max_chunk_nb, P], gdt, tag="gb")
                        ni = ck.nblk * P
                        nc.gpsimd.dma_gather(
                            gb[:, :ck.nblk, :], h_sec[li][ck.half][:],
                            idx_sb[li][:, ck.col0:ck.col0 + ni // 16],
                            ni, ni, P, single_packet=False,
                            queue_num=qctr[0] % N_QUEUES)
                        qctr[0] += 1
                        gbufs[ci] = gb

                    tts = sorted(grp.tiles.keys())
                    aggp = None
                    for k, tt in enumerate(tts):
                        if k % 4 == 0:
                            aggp = aggpool.tile([P, 512], f32, tag="aggp")
                        sl = slice((k % 4) * P, (k % 4) * P + P)
                        segs = grp.tiles[tt]
                        nb_tot = sum(nb for (_, _, nb) in segs) + 1
                        bi = 0
                        for (ci, j0, nb) in segs:
                            ck = lay.chunks[ci]
                            gb = gbufs[ci]
                            for j in range(j0, j0 + nb):
                                b = ck.blk0 + j
                                nc.tensor.matmul(
                                    out=aggp[:, sl], lhsT=gb[:, j, :],
                                    rhs=mw[:, (b - gb0) * P:(b - gb0 + 1) * P],
                                    start=(bi == 0), stop=False)
                                bi += 1
                        # self-loop block: own-shard H rows, scaled by dis^2
                        rows = slice(tt * P, (tt + 1) * P)
                        gs = spool.tile([P, P], gdt, tag="gself")
                        nc.sync.dma_start(gs[:], h_shard[li][rows, :])
                        gss = spool.tile([P, P], gdt, tag="gselfs")
                        nc.scalar.activation(out=gss[:], in_=gs[:], func=copyf,
                                             scale=sw_sb[li][:, tt:tt + 1])
                        nc.tensor.matmul(out=aggp[:, sl], lhsT=gss[:],
                                         rhs=ident_sb[:], start=(bi == 0),
                                         stop=True)
                        # relu(agg + b) in transposed layout (bias per-part)
                        outT = opool.tile([P, P], f32, tag="outT")
                        nc.scalar.activation(out=outT[:], in_=aggp[:, sl],
                                             func=relu, bias=bcol[:])
                        hp2 = hpool.tile([P, n_next], f32, tag="hpsum")
                        nc.tensor.matmul(out=hp2[:], lhsT=outT[:],
                                         rhs=w_next[:], start=True, stop=True)
                        if li == 0:
                            hsb = opool.tile([P, n_next], gdt, tag="hsb")
                            nc.scalar.activation(out=hsb[:], in_=hp2[:],
                                                 func=copyf)
                            nc.sync.dma_start(h_shard[1][rows, :], hsb[:])
                            for s in range(N_SEC):
                                if tt * P < (s + 1) * sec_rows <= (tt + 1) * P:
                                    nc.gpsimd.collective_compute(
                                        "AllGather", mybir.AluOpType.bypass,
                                        replica_groups=rg,
                                        ins=[h_shard[1][s * sec_rows:
                                                        (s + 1) * sec_rows, :]],
                                        outs=[h_sec[1][s][:]])
                        else:
                            osb = opool.tile([P, F_OUT], f32, tag="osb")
                            nc.vector.tensor_tensor(
                                out=osb[:], in0=hp2[:], in1=blbc_sb[:],
                                op=mybir.AluOpType.add)
                            nc.sync.dma_start(out_d[rows, :], osb[:])

    nc.compile()
    return nc


# ---------------------------------------------------------------------------
# Entry point
# ---------------------------------------------------------------------------

def _run(x, edge_index, mask1, mask2, W1, b1, W2, b2, Wl, bl,
         n, n_pad, lo_limit=None):
    layouts, in_maps = _prepare(x, edge_index, mask1, mask2,
                                W1, b1, W2, b2, Wl, bl, n, n_pad)
    nc = _build(layouts, n_pad)
    res = run_bass_kernel_spmd(nc, in_maps, core_ids=list(range(N_CORES)))
    out = np.concatenate([res.results[c]["out"] for c in range(N_CORES)],
                         axis=0)
    return out[:n].astype(np.float32)


def kernel(x, edge_index, mask1, mask2, W1, b1, W2, b2, Wl, bl):
    n_pad = 50176  # 8 cores * 49 tiles * 128
    return _run(x, edge_index, mask1, mask2, W1, b1, W2, b2, Wl, bl,
                N_NODES, n_pad)

